# revision 16
# baseline (speedup 1.0000x reference)
"""AVWGCN (adaptive vertex-wise graph convolution) Trainium2 kernel.

Reference computation (per batch b):
  bias = STE @ bias_pool                               [n, o]
  T0 = SC, T1 = R, T2 = 2 R@R - SC                     (Chebyshev, K=3)
  h_k = T_k @ x                                        [n, k, i]
  z   = einsum('nki,dkio->ndo', h, weights_pool)
  out = einsum('ndo,nd->no', z, STE) + bias

Key algebraic restructure: T2 is only used via T2 @ x, so
  h_2 = 2 R @ (R @ x) - SC @ x = 2 R @ h_1 - h_0
which avoids the O(N^3) matmul entirely.

Sharding: data-parallel over batch, 4 batches per core across 8 cores.
All matmuls run in float32r (full-rate fp32 PE mode, ~1e-4 rel err).

Layout notes (per core):
  - PE matmuls contract the partition dim of both operands, so the graph
    matrices are needed with m (their column index) on partitions; the
    per-core shards of R/SC are uploaded in that (transposed) layout.
  - h is produced directly in transposed layout hT[(k,i), pos] via
    out = lhsT.T @ rhs with lhsT = x (so no transpose of h needed for z).
  - z psum tiles are [pos=128, (o,d)=1024] halves (d innermost); the STE
    contraction over d runs as ONE custom fused DVE op per half:
    prefix-sum-of-products (scan) whose output AP collapses the d axis
    (stride-0 write), leaving cumulative group ends; a single shifted
    subtract per batch turns those into the final grouped sums.
  - bias is folded into z as an extra contraction row: hT_b carries a
    constant-ones row 64, and W2b carries bias_pool as row 64.
"""

import sys

sys.path.insert(0, "/opt/trn_rl_repo")

import numpy as np

import concourse.bacc as bacc
import concourse.mybir as mybir
import concourse.tile as tile
from concourse import bass_utils
from concourse import dve_ops as _dv
from concourse.dve_spec import Spec, Src0, Src1, scan, AluOp, lower
from concourse.dve_ops import DveOp, OPS
from concourse.dve_uop import DveOpSpec

F32R = mybir.dt.float32r
F32 = mybir.dt.float32

B, N, DIN, DOUT, CHEB_K, ET = 32, 512, 64, 64, 3, 32
N_CORES = 8
B_PER_CORE = B // N_CORES  # 4
P = 128
S = N // P  # 4 pos-chunks per batch
DO = DOUT * ET  # 2048
HALF_O = DOUT // 2  # 32 o-values per psum half

_cached = {}


def _mulscan_op():
    """out = prefix_sum(in0 * in1) along the free stream (fp32 accum)."""
    if "mulscan" in _cached:
        return _cached["mulscan"]

    def _ref(in0, in1, s0, s1, imm2):
        prod = in0.astype(np.float32) * in1.astype(np.float32)
        flat = np.cumsum(prod.reshape(prod.shape[0], -1), axis=1)
        return flat.reshape(prod.shape).astype(np.float32)

    spec = Spec(body=scan(AluOp.ADD, Src0 * Src1), reference=_ref)
    shas = {}
    for ver in ("v3", "v4"):
        s = DveOpSpec(name="MULSCAN_ANT", opcode=0, uops=lower(spec, ver=ver), rd1_en=True)
        shas[ver] = s.sha(ver)
    op = DveOp("MULSCAN_ANT", spec, subdim=False, uops_sha=shas)
    OPS.append(op)
    _dv._SUB_OPCODE_FOR_NAME[op.name] = _dv._CUSTOM_DVE_ROW_BASE + len(OPS) - 1
    _dv.CUSTOM_DVE_SPECS[op.name] = op.spec
    _cached["mulscan"] = op
    return op


def _build_kernel():
    MULSCAN = _mulscan_op()
    nc = bacc.Bacc("TRN2", target_bir_lowering=False)

    # RT/SCT are the per-core R/SC shards in [b, m, n] (transposed) layout.
    RT_d = nc.dram_tensor("RT", [B_PER_CORE, N, N], F32R, kind="ExternalInput")
    SCT_d = nc.dram_tensor("SCT", [B_PER_CORE, N, N], F32R, kind="ExternalInput")
    x_d = nc.dram_tensor("x", [B_PER_CORE, N, DIN], F32R, kind="ExternalInput")
    STE_d = nc.dram_tensor("STE", [B_PER_CORE, N, ET], F32, kind="ExternalInput")
    # W2a: rows ki=0..127 (k=0,1); W2b: rows ki=128..191 (k=2) + bias row.
    # columns ordered (o, d): col = o*ET + d.
    W2a_d = nc.dram_tensor("W2a", [P, DO], F32R, kind="ExternalInput")
    W2b_d = nc.dram_tensor("W2b", [DIN + 1, DO], F32R, kind="ExternalInput")
    I128_d = nc.dram_tensor("I128", [P, P], F32R, kind="ExternalInput")
    nhI_d = nc.dram_tensor("nhI", [DIN, DIN], F32R, kind="ExternalInput")  # -0.5*I
    out_d = nc.dram_tensor("out", [B_PER_CORE, N, DOUT], F32, kind="ExternalOutput")

    with tile.TileContext(nc) as tc:
        with (
            tc.tile_pool(name="const", bufs=1) as cpool,
            tc.tile_pool(name="load", bufs=2) as lpool,
            tc.tile_pool(name="work", bufs=2) as wpool,
            tc.tile_pool(name="psh", bufs=2, space="PSUM") as psh,
            tc.tile_pool(name="psz", bufs=2, space="PSUM") as psz,
        ):
            w2a = cpool.tile([P, DO], F32R)
            w2b = cpool.tile([DIN + 1, DO], F32R)
            ident = cpool.tile([P, P], F32R)
            nhI = cpool.tile([DIN, DIN], F32R)
            # Small consts ride the gpsimd (SWDGE) path; weights are issued
            # on sync after batch 0's loads (needed only by the z matmuls).
            nc.gpsimd.dma_start(ident[:], I128_d[:])
            nc.gpsimd.dma_start(nhI[:], nhI_d[:])

            for b in range(B_PER_CORE):
                # ---- loads (RT/SCT stripes: [p=m%128, s=m//128, n]) ----
                # Per-stripe DMAs so the first h matmul only waits for one
                # 256KB stripe; SCT/xb on the sync HWDGE ring, RT/steb on
                # the (otherwise idle) gpsimd SWDGE queue so the scalar
                # engine stays free for PSUM->SBUF copies.
                RT = lpool.tile([P, S, N], F32R, tag="RT")
                SCT = lpool.tile([P, S, N], F32R, tag="SCT")
                xb = lpool.tile([P, S, DIN], F32R, tag="xb")
                steb = lpool.tile([P, S, ET], F32, tag="steb")
                nc.sync.dma_start(xb[:], x_d[b].rearrange("(s p) i -> p s i", p=P))
                nc.gpsimd.dma_start(
                    steb[:], STE_d[b].rearrange("(s p) d -> p s d", p=P)
                )
                nc.sync.dma_start(SCT[:], SCT_d[b].rearrange("(s p) n -> p s n", p=P))
                nc.gpsimd.dma_start(RT[:], RT_d[b].rearrange("(s p) n -> p s n", p=P))
                if b == 0:
                    nc.sync.dma_start(w2a[:], W2a_d[:])
                    nc.sync.dma_start(w2b[:], W2b_d[:])

                # ---- h matmuls ----
                hT_a = wpool.tile([P, N], F32R, tag="hT_a")
                ph0 = psh.tile([DIN, N], F32, tag="ph0")
                ph1 = psh.tile([DIN, N], F32, tag="ph1")
                for s in range(S):
                    nc.tensor.matmul(
                        ph0[:], xb[:, s, :], SCT[:, s, :],
                        start=(s == 0), stop=(s == S - 1),
                    )
                for s in range(S):
                    nc.tensor.matmul(
                        ph1[:], xb[:, s, :], RT[:, s, :],
                        start=(s == 0), stop=(s == S - 1),
                    )
                nc.scalar.copy(hT_a[0:DIN, :], ph0[:])
                nc.scalar.copy(hT_a[DIN : 2 * DIN, :], ph1[:])

                # h1 natural [m, i] via PE transpose of h1T
                ph1n = psh.tile([P, S * DIN], F32R, tag="ph1")
                for c in range(S):
                    nc.tensor.transpose(
                        ph1n[:, DIN * c : DIN * (c + 1)],
                        hT_a[DIN : 2 * DIN, P * c : P * (c + 1)],
                        ident[DIN : 2 * DIN, DIN : 2 * DIN],
                    )
                h1n = wpool.tile([P, S, DIN], F32R, tag="h1n")
                nc.scalar.copy(h1n[:], ph1n[:].rearrange("p (s i) -> p s i", i=DIN))

                # hT_b [65, n]: rows 0-63 = h2T = (2 R@h1 - h0).T, row 64 = ones
                ph2 = psh.tile([DIN, N], F32, tag="ph0")
                for s in range(S):
                    nc.tensor.matmul(
                        ph2[:], h1n[:, s, :], RT[:, s, :],
                        start=(s == 0), stop=False,
                    )
                nc.tensor.matmul(ph2[:], nhI[:], hT_a[0:DIN, :], start=False, stop=True)
                hT_b = wpool.tile([DIN + 1, N], F32R, tag="hT_b")
                nc.scalar.mul(hT_b[0:DIN, :], ph2[:], 2.0)
                nc.vector.memset(hT_b[DIN : DIN + 1, :].bitcast(F32), 1.0)

                # ---- z matmuls + fused STE contraction ----
                E = wpool.tile([P, S, DOUT], F32, tag="E")
                for s in range(S):
                    for half in range(2):
                        pz = psz.tile([P, DO // 2], F32, tag="pz")
                        for jj in range(2):
                            nsl_lo = 1024 * half + 512 * jj
                            nsl = slice(nsl_lo, nsl_lo + 512)
                            psl = slice(512 * jj, 512 * (jj + 1))
                            nc.tensor.matmul(
                                pz[:, psl],
                                hT_a[:, P * s : P * (s + 1)],
                                w2a[:, nsl],
                                start=True, stop=False,
                            )
                        for jj in range(2):
                            nsl_lo = 1024 * half + 512 * jj
                            nsl = slice(nsl_lo, nsl_lo + 512)
                            psl = slice(512 * jj, 512 * (jj + 1))
                            nc.tensor.matmul(
                                pz[:, psl],
                                hT_b[:, P * s : P * (s + 1)],
                                w2b[:, nsl],
                                start=False, stop=True,
                            )
                        ste_b = (
                            steb[:, s, :]
                            .unsqueeze(1)
                            .broadcast_to([P, HALF_O, ET])
                        )
                        eslice = E[:, s, HALF_O * half : HALF_O * (half + 1)]
                        nc.vector._custom_dve(
                            MULSCAN,
                            out=eslice.unsqueeze(2).broadcast_to([P, HALF_O, ET]),
                            in0=pz[:].rearrange("p (o d) -> p o d", d=ET),
                            in1=ste_b,
                        )

                # grouped sums: out = E[g] - E[g-1] within each 32-wide half
                outb = wpool.tile([P, S, DOUT], F32, tag="outb")
                E_v = E[:].rearrange("p s (h g) -> p (s h) g", g=HALF_O)
                o_v = outb[:].rearrange("p s (h g) -> p (s h) g", g=HALF_O)
                nc.vector.tensor_copy(o_v[:, :, 0:1], E_v[:, :, 0:1])
                nc.vector.tensor_tensor(
                    o_v[:, :, 1:HALF_O],
                    E_v[:, :, 1:HALF_O],
                    E_v[:, :, 0 : HALF_O - 1],
                    op=mybir.AluOpType.subtract,
                )

                nc.sync.dma_start(
                    out_d[b].rearrange("(s p) o -> p s o", p=P), outb[:]
                )

    nc.compile()
    return nc


def _prep_consts(weights_pool, bias_pool):
    # W2 columns ordered (o, d): col = o*ET + d; rows ki = k*DIN + i.
    w2 = np.ascontiguousarray(
        weights_pool.transpose(1, 2, 3, 0).reshape(CHEB_K * DIN, DO)
    ).astype(np.float32)
    w2a = w2[:P]
    w2b = np.concatenate(
        [w2[P:], bias_pool.T.reshape(1, DO).astype(np.float32)], axis=0
    )
    i128 = np.eye(P, dtype=np.float32)
    nhI = (-0.5 * np.eye(DIN)).astype(np.float32)
    return np.ascontiguousarray(w2a), np.ascontiguousarray(w2b), i128, nhI


def kernel(x, STE, R, SC, weights_pool, bias_pool, _trace=False):
    x = np.asarray(x, dtype=np.float32)
    STE = np.asarray(STE, dtype=np.float32)
    R = np.asarray(R, dtype=np.float32)
    SC = np.asarray(SC, dtype=np.float32)
    weights_pool = np.asarray(weights_pool, dtype=np.float32)
    bias_pool = np.asarray(bias_pool, dtype=np.float32)

    if "nc" not in _cached:
        _cached["nc"] = _build_kernel()
    nc = _cached["nc"]

    w2a, w2b, i128, nhI = _prep_consts(weights_pool, bias_pool)
    # Upload the per-core R/SC shards with the graph-node axis to be
    # contracted (m) leading, i.e. transposed per batch.
    RT_all = np.ascontiguousarray(R.transpose(0, 2, 1))
    SCT_all = np.ascontiguousarray(SC.transpose(0, 2, 1))
    in_maps = []
    for c in range(N_CORES):
        lo, hi = c * B_PER_CORE, (c + 1) * B_PER_CORE
        in_maps.append(
            {
                "RT": RT_all[lo:hi],
                "SCT": SCT_all[lo:hi],
                "x": np.ascontiguousarray(x[lo:hi]),
                "STE": np.ascontiguousarray(STE[lo:hi]),
                "W2a": w2a,
                "W2b": w2b,
                "I128": i128,
                "nhI": nhI,
            }
        )

    res = bass_utils.run_bass_kernel_spmd(
        nc, in_maps, core_ids=list(range(N_CORES)), trace=_trace
    )
    out = np.concatenate([r["out"] for r in res.results], axis=0)
    if _trace:
        kernel.last_result = res
    return out
